# revision 2
# baseline (speedup 1.0000x reference)
"""CenterLoss kernel for 8 Trainium2 NeuronCores.

The reference discards the addmm cross term, so

    loss = (1/B) * sum_i (||x_i||^2 + ||centers[y_i]||^2) + (C-1) * 1e-12

(the constant comes from clip(0, 1e-12, 1e12) applied to the B*(C-1) zero
entries of dist; the nonzero entries are ~1e2, far inside the clamp range).
The [B, C] matrix never needs to be materialized.

Data-parallel over batch: each core squares+row-sums its x slab (viewed as
[128, 2048]) and indirect-DMA gathers its 2048 centers[y_i] rows (16 gathers
of 128 rows, one row per partition), squares+row-sums those, and emits a
[128, 1] partial. The host sums the 8x128 partials in float64.

Raw Bass (no Tile): this toolchain's walrus rejects Tile's tail drain and
TENSOR_TENSOR_REDUCE, so sync is explicit. DVE same-engine RAW needs drain().
Each DMA gets its own semaphore (a shared counting sem across in-flight DMAs
is racy: the 16 per-SDMA-engine increments of different DMAs interleave).
"""

import numpy as np

import concourse.bass as bass
from concourse import mybir
from concourse.bass_utils import run_bass_kernel_spmd

B = 16384  # batch
D = 128  # feature dim
C = 10000  # num classes
M = 8  # cores
P = 128  # SBUF partitions
BPC = B // M  # 2048 batch rows per core
XFREE = BPC * D // P  # 2048 f32 per partition when x slab is viewed [P, -1]
NG = BPC // P  # 16 gather groups of 128 indices
XCH = 4  # x chunks for DMA/compute overlap
XW = XFREE // XCH

_F32 = mybir.dt.float32
_I32 = mybir.dt.int32


def _build_nc() -> bass.Bass:
    nc = bass.Bass()
    xs = nc.dram_tensor("xs", [P, XFREE], _F32, kind="ExternalInput")
    ys = nc.dram_tensor("ys", [P, NG], _I32, kind="ExternalInput")
    cs = nc.dram_tensor("cs", [C, D], _F32, kind="ExternalInput")
    out = nc.dram_tensor("out", [P, 1], _F32, kind="ExternalOutput")

    with (
        nc.sbuf_tensor([P, NG], _I32) as y_sb,
        nc.sbuf_tensor([P, NG, D], _F32) as g_sb,
        nc.sbuf_tensor([P, XCH, XW], _F32) as x_sb,
        nc.sbuf_tensor([P, XCH, XW], _F32) as tr_sb,
        nc.sbuf_tensor([P, NG, D], _F32) as trg_sb,
        nc.sbuf_tensor([P, 8], _F32) as acc_sb,
        nc.semaphore() as ysem,
        nc.semaphore() as xsem0,
        nc.semaphore() as xsem1,
        nc.semaphore() as xsem2,
        nc.semaphore() as xsem3,
        nc.semaphore() as gsem,
        nc.semaphore() as vsem,
        nc.semaphore() as osem,
        nc.Block() as block,
    ):
        xsems = [xsem0, xsem1, xsem2, xsem3]

        @block.sync
        def _(sync):
            sync.dma_start(out=y_sb[:], in_=ys[:]).then_inc(ysem, 16)
            for i in range(XCH):
                sync.dma_start(
                    out=x_sb[:, i, :], in_=xs[:, i * XW : (i + 1) * XW]
                ).then_inc(xsems[i], 16)
            sync.wait_ge(vsem, 1)
            sync.dma_start(out=out[:], in_=acc_sb[:, 7:8]).then_inc(osem, 16)
            sync.wait_ge(osem, 16)

        @block.gpsimd
        def _(gpsimd):
            gpsimd.wait_ge(ysem, 16)
            for g in range(NG):
                gpsimd.indirect_dma_start(
                    out=g_sb[:, g, :],
                    out_offset=None,
                    in_=cs[:],
                    in_offset=bass.IndirectOffsetOnAxis(ap=y_sb[:, g : g + 1], axis=0),
                ).then_inc(gsem, 16)

        @block.vector
        def _(vector):
            for i in range(XCH):
                vector.wait_ge(xsems[i], 16)
                vector.tensor_mul(tr_sb[:, i, :], x_sb[:, i, :], x_sb[:, i, :])
            vector.wait_ge(gsem, 16 * NG)
            g_flat = g_sb[:].rearrange("p a b -> p (a b)")
            trg_flat = trg_sb[:].rearrange("p a b -> p (a b)")
            vector.tensor_mul(trg_flat, g_flat, g_flat)
            vector.drain()
            for i in range(XCH):
                vector.reduce_sum(
                    acc_sb[:, i : i + 1], tr_sb[:, i, :], axis=mybir.AxisListType.X
                )
            vector.reduce_sum(acc_sb[:, 4:5], trg_flat, axis=mybir.AxisListType.X)
            vector.memset(acc_sb[:, 5:7], 0.0)
            vector.drain()
            vector.reduce_sum(acc_sb[:, 7:8], acc_sb[:, 0:7], axis=mybir.AxisListType.X)
            vector.drain()
            vector.engine_nop().then_inc(vsem, 1)

    return nc


_NC_CACHE: list = []


def _get_nc() -> bass.Bass:
    if not _NC_CACHE:
        _NC_CACHE.append(_build_nc())
    return _NC_CACHE[0]


def _in_maps(x: np.ndarray, centers: np.ndarray, y: np.ndarray) -> list[dict]:
    x = np.ascontiguousarray(np.asarray(x, dtype=np.float32))
    centers = np.ascontiguousarray(np.asarray(centers, dtype=np.float32))
    y32 = np.ascontiguousarray(np.asarray(y).astype(np.int32))
    maps = []
    for k in range(M):
        maps.append(
            {
                "xs": x[k * BPC : (k + 1) * BPC].reshape(P, XFREE),
                "ys": y32[k * BPC : (k + 1) * BPC].reshape(P, NG),
                "cs": centers,
            }
        )
    return maps


def _finalize(results: list[dict]) -> np.ndarray:
    total = 0.0
    for r in results:
        total += float(np.sum(r["out"].astype(np.float64)))
    loss = total / B + (C - 1) * 1e-12
    return np.float32(loss)


def run(x, centers, y, **spmd_kwargs):
    """Run on 8 cores; returns (loss, BassKernelResults)."""
    nc = _get_nc()
    res = run_bass_kernel_spmd(nc, _in_maps(x, centers, y), list(range(M)), **spmd_kwargs)
    return _finalize(res.results), res


def kernel(x: np.ndarray, centers: np.ndarray, y: np.ndarray) -> np.ndarray:
    loss, _ = run(x, centers, y)
    return loss


# revision 3
# speedup vs baseline: 1.0506x; 1.0506x over previous
"""CenterLoss kernel for 8 Trainium2 NeuronCores.

The reference discards the addmm cross term, so

    loss = (1/B) * sum_i (||x_i||^2 + ||centers[y_i]||^2) + (C-1) * 1e-12

(the constant comes from clip(0, 1e-12, 1e12) applied to the B*(C-1) zero
entries of dist; the nonzero entries are ~1e2, far inside the clamp range).
The [B, C] matrix never needs to be materialized.

Data-parallel over batch: each core squares+row-sums its x slab (viewed as
[128, 2048]) and indirect-DMA gathers its 2048 centers[y_i] rows (16 gathers
of 128 rows, one row per partition), squares+row-sums those, and emits a
[128, 1] partial. The host sums the 8x128 partials in float64.

Raw Bass (no Tile): this toolchain's walrus rejects Tile's tail drain,
TENSOR_TENSOR_REDUCE, and PseudoReloadLibraryIndex (so no dma_gather), and
multi-offset indirect DMAs corrupt ~0.5% of rows; sync is explicit. DVE
same-engine RAW needs drain(). A semaphore may only be waited at its full
count (partial counts race: the 16 per-SDMA-engine increments of concurrent
DMAs interleave), hence one sem per x/y DMA and per 4-gather group.
"""

import numpy as np

import concourse.bass as bass
from concourse import mybir
from concourse.bass_utils import run_bass_kernel_spmd

B = 16384  # batch
D = 128  # feature dim
C = 10000  # num classes
M = 8  # cores
P = 128  # SBUF partitions
BPC = B // M  # 2048 batch rows per core
XFREE = BPC * D // P  # 2048 f32 per partition when x slab is viewed [P, -1]
NG = BPC // P  # 16 gather groups of 128 indices
NGRP = 4  # gather semaphore groups
GW = NG // NGRP

_F32 = mybir.dt.float32
_I32 = mybir.dt.int32


def _build_nc() -> bass.Bass:
    nc = bass.Bass()
    xs = nc.dram_tensor("xs", [P, XFREE], _F32, kind="ExternalInput")
    ys = nc.dram_tensor("ys", [P, NG], _I32, kind="ExternalInput")
    cs = nc.dram_tensor("cs", [C, D], _F32, kind="ExternalInput")
    out = nc.dram_tensor("out", [P, 1], _F32, kind="ExternalOutput")

    with (
        nc.sbuf_tensor([P, NG], _I32) as y_sb,
        nc.sbuf_tensor([P, NG, D], _F32) as g_sb,
        nc.sbuf_tensor([P, XFREE], _F32) as x_sb,
        nc.sbuf_tensor([P, XFREE], _F32) as tr_sb,
        nc.sbuf_tensor([P, NG, D], _F32) as trg_sb,
        nc.sbuf_tensor([P, 6], _F32) as acc_sb,
        nc.semaphore() as ysem,
        nc.semaphore() as xsem,
        nc.semaphore() as gsem0,
        nc.semaphore() as gsem1,
        nc.semaphore() as gsem2,
        nc.semaphore() as gsem3,
        nc.semaphore() as vsem,
        nc.semaphore() as osem,
        nc.Block() as block,
    ):
        gsems = [gsem0, gsem1, gsem2, gsem3]

        @block.sync
        def _(sync):
            sync.dma_start(out=x_sb[:], in_=xs[:]).then_inc(xsem, 16)
            sync.wait_ge(vsem, 1)
            sync.dma_start(out=out[:], in_=acc_sb[:, 5:6]).then_inc(osem, 16)
            sync.wait_ge(osem, 16)

        @block.gpsimd
        def _(gpsimd):
            gpsimd.dma_start(out=y_sb[:], in_=ys[:]).then_inc(ysem, 16)
            gpsimd.wait_ge(ysem, 16)
            for g in range(NG):
                gpsimd.indirect_dma_start(
                    out=g_sb[:, g, :],
                    out_offset=None,
                    in_=cs[:],
                    in_offset=bass.IndirectOffsetOnAxis(ap=y_sb[:, g : g + 1], axis=0),
                ).then_inc(gsems[g // GW], 16)

        @block.vector
        def _(vector):
            vector.wait_ge(xsem, 16)
            vector.tensor_mul(tr_sb[:], x_sb[:], x_sb[:])
            for k in range(NGRP):
                vector.wait_ge(gsems[k], 16 * GW)
                sl = slice(k * GW, (k + 1) * GW)
                gf = g_sb[:, sl, :].rearrange("p a b -> p (a b)")
                tf = trg_sb[:, sl, :].rearrange("p a b -> p (a b)")
                vector.tensor_mul(tf, gf, gf)
                vector.drain()
                if k == 0:
                    vector.reduce_sum(
                        acc_sb[:, 0:1], tr_sb[:], axis=mybir.AxisListType.X
                    )
                vector.reduce_sum(
                    acc_sb[:, 1 + k : 2 + k], tf, axis=mybir.AxisListType.X
                )
            vector.drain()
            vector.reduce_sum(acc_sb[:, 5:6], acc_sb[:, 0:5], axis=mybir.AxisListType.X)
            vector.drain()
            vector.nop().then_inc(vsem, 1)

    return nc



_NC_CACHE: list = []


def _get_nc() -> bass.Bass:
    if not _NC_CACHE:
        _NC_CACHE.append(_build_nc())
    return _NC_CACHE[0]


def _in_maps(x: np.ndarray, centers: np.ndarray, y: np.ndarray) -> list[dict]:
    x = np.ascontiguousarray(np.asarray(x, dtype=np.float32))
    centers = np.ascontiguousarray(np.asarray(centers, dtype=np.float32))
    y32 = np.ascontiguousarray(np.asarray(y).astype(np.int32))
    maps = []
    for k in range(M):
        maps.append(
            {
                "xs": x[k * BPC : (k + 1) * BPC].reshape(P, XFREE),
                "ys": y32[k * BPC : (k + 1) * BPC].reshape(P, NG),
                "cs": centers,
            }
        )
    return maps


def _finalize(results: list[dict]) -> np.ndarray:
    total = 0.0
    for r in results:
        total += float(np.sum(r["out"].astype(np.float64)))
    loss = total / B + (C - 1) * 1e-12
    return np.float32(loss)


def run(x, centers, y, **spmd_kwargs):
    """Run on 8 cores; returns (loss, BassKernelResults)."""
    nc = _get_nc()
    res = run_bass_kernel_spmd(nc, _in_maps(x, centers, y), list(range(M)), **spmd_kwargs)
    return _finalize(res.results), res


def kernel(x: np.ndarray, centers: np.ndarray, y: np.ndarray) -> np.ndarray:
    loss, _ = run(x, centers, y)
    return loss


# revision 4
# speedup vs baseline: 1.1737x; 1.1172x over previous
"""CenterLoss kernel for 8 Trainium2 NeuronCores.

The reference discards the addmm cross term, so

    loss = (1/B) * sum_i (||x_i||^2 + ||centers[y_i]||^2) + (C-1) * 1e-12

(the constant comes from clip(0, 1e-12, 1e12) applied to the B*(C-1) zero
entries of dist; the nonzero entries are ~1e2, far inside the clamp range).
The [B, C] matrix never needs to be materialized.

Data-parallel over batch: each core squares+row-sums its x slab (viewed as
[128, 2048]) and gathers its 2048 centers[y_i] rows with 4 dma_gather
instructions (512 rows each; 2048 in one overflows the SWDGE ring and
kills the NEFF), squares+row-sums those, collapses the [128,1] partial to
one element with a PE matmul against ones (a [128,1]-strided output DMA
costs ~8us in completion latency; a 1-element DMA doesn't), and writes a
scalar. The host sums the 8 partials in float64.

Raw Bass (no Tile): this toolchain's walrus rejects Tile's tail drain and
runtime-faults on TENSOR_TENSOR_REDUCE, so sync is explicit. Extended
bass_isa instructions (dma_gather, load_library) need
library_overlay.lower_extended_insts(nc) or walrus sees empty .instr bytes
("ISA wrong length"). dma_gather needs gpsimd.load_library(mlp). DVE
same-engine RAW needs drain(). A semaphore may only be waited at its full
count (partial counts race: the 16 per-SDMA-engine increments of
concurrent DMAs interleave), hence one sem per DMA / gather chunk.
"""

import numpy as np

import concourse.bass as bass
from concourse import library_config, library_overlay, mybir
from concourse.bass_utils import run_bass_kernel_spmd

B = 16384  # batch
D = 128  # feature dim
C = 10000  # num classes
M = 8  # cores
P = 128  # SBUF partitions
BPC = B // M  # 2048 batch rows per core
XFREE = BPC * D // P  # 2048 f32 per partition when x slab is viewed [P, -1]
NG = BPC // P  # 16 gathered-row groups of 128 (dest layout [P, NG, D])
NCHUNK = 4  # dma_gather instructions per core
IDXW = BPC // 16  # idx columns in the 16-partition-wrapped int16 layout

_F32 = mybir.dt.float32
_I16 = mybir.dt.int16


def _build_nc() -> bass.Bass:
    nc = bass.Bass()
    xs = nc.dram_tensor("xs", [P, XFREE], _F32, kind="ExternalInput")
    ys = nc.dram_tensor("ys", [P, IDXW], _I16, kind="ExternalInput")
    cs = nc.dram_tensor("cs", [C, D], _F32, kind="ExternalInput")
    out = nc.dram_tensor("out", [1, 1], _F32, kind="ExternalOutput")

    with (
        nc.sbuf_tensor([P, IDXW], _I16) as y_sb,
        nc.sbuf_tensor([P, NG, D], _F32) as g_sb,
        nc.sbuf_tensor([P, XFREE], _F32) as x_sb,
        nc.sbuf_tensor([P, XFREE], _F32) as tr_sb,
        nc.sbuf_tensor([P, NG, D], _F32) as trg_sb,
        nc.sbuf_tensor([P, 6], _F32) as acc_sb,
        nc.sbuf_tensor([P, 1], _F32) as ones_sb,
        nc.sbuf_tensor([1, 1], _F32) as fin_sb,
        nc.psum_tensor([1, 1], _F32) as ps,
        nc.semaphore() as ysem,
        nc.semaphore() as xsem,
        nc.semaphore() as gsem0,
        nc.semaphore() as gsem1,
        nc.semaphore() as gsem2,
        nc.semaphore() as gsem3,
        nc.semaphore() as vsem,
        nc.semaphore() as tsem,
        nc.semaphore() as csem,
        nc.semaphore() as osem,
        nc.Block() as block,
    ):
        gsems = [gsem0, gsem1, gsem2, gsem3]

        @block.sync
        def _(sync):
            sync.dma_start(out=x_sb[:], in_=xs[:]).then_inc(xsem, 16)
            sync.wait_ge(csem, 1)
            sync.dma_start(out=out[:], in_=fin_sb[:]).then_inc(osem, 16)
            sync.wait_ge(osem, 16)

        @block.gpsimd
        def _(gpsimd):
            gpsimd.dma_start(out=y_sb[:], in_=ys[:]).then_inc(ysem, 16)
            gpsimd.load_library(library_config.mlp)
            gpsimd.wait_ge(ysem, 16)
            for c in range(NCHUNK):
                nidx = BPC // NCHUNK  # 512 rows per gather
                gpsimd.dma_gather(
                    out_ap=g_sb[:, c * (NG // NCHUNK) : (c + 1) * (NG // NCHUNK), :],
                    in_ap=cs[:],
                    idxs_ap=y_sb[:, c * (IDXW // NCHUNK) : (c + 1) * (IDXW // NCHUNK)],
                    num_idxs=nidx,
                    num_idxs_reg=nidx,
                    elem_size=D,
                ).then_inc(gsems[c], 16)

        @block.vector
        def _(vector):
            vector.memset(ones_sb[:], 1.0)
            vector.wait_ge(xsem, 16)
            vector.tensor_mul(tr_sb[:], x_sb[:], x_sb[:])
            for k in range(NCHUNK):
                vector.wait_ge(gsems[k], 16)
                sl = slice(k * (NG // NCHUNK), (k + 1) * (NG // NCHUNK))
                gf = g_sb[:, sl, :].rearrange("p a b -> p (a b)")
                tf = trg_sb[:, sl, :].rearrange("p a b -> p (a b)")
                vector.tensor_mul(tf, gf, gf)
                vector.drain()
                if k == 0:
                    vector.reduce_sum(
                        acc_sb[:, 0:1], tr_sb[:], axis=mybir.AxisListType.X
                    )
                vector.reduce_sum(
                    acc_sb[:, 1 + k : 2 + k], tf, axis=mybir.AxisListType.X
                )
            vector.drain()
            vector.reduce_sum(acc_sb[:, 5:6], acc_sb[:, 0:5], axis=mybir.AxisListType.X)
            vector.drain()
            vector.nop().then_inc(vsem, 1)
            vector.wait_ge(tsem, 1)
            vector.tensor_copy(fin_sb[:], ps[:])
            vector.drain()
            vector.nop().then_inc(csem, 1)

        @block.tensor
        def _(tensor):
            tensor.wait_ge(vsem, 1)
            nc.tensor.matmul(
                ps[:], lhsT=acc_sb[:, 5:6], rhs=ones_sb[:], start=True, stop=True
            ).then_inc(tsem, 1)

    library_overlay.lower_extended_insts(nc)
    return nc


_NC_CACHE: list = []


def _get_nc() -> bass.Bass:
    if not _NC_CACHE:
        _NC_CACHE.append(_build_nc())
    return _NC_CACHE[0]


def _in_maps(x: np.ndarray, centers: np.ndarray, y: np.ndarray) -> list[dict]:
    x = np.ascontiguousarray(np.asarray(x, dtype=np.float32))
    centers = np.ascontiguousarray(np.asarray(centers, dtype=np.float32))
    y64 = np.asarray(y).reshape(B)
    maps = []
    for k in range(M):
        y_slab = y64[k * BPC : (k + 1) * BPC].astype(np.int16)
        # 16-partition-wrapped dma_gather layout, replicated to all 8 Q7 cores
        blk = y_slab.reshape(IDXW, 16).T  # [16, IDXW]
        maps.append(
            {
                "xs": x[k * BPC : (k + 1) * BPC].reshape(P, XFREE),
                "ys": np.ascontiguousarray(np.tile(blk, (8, 1))),  # [128, IDXW]
                "cs": centers,
            }
        )
    return maps


def _finalize(results: list[dict]) -> np.ndarray:
    total = 0.0
    for r in results:
        total += float(np.sum(r["out"].astype(np.float64)))
    loss = total / B + (C - 1) * 1e-12
    return np.float32(loss)


def run(x, centers, y, **spmd_kwargs):
    """Run on 8 cores; returns (loss, BassKernelResults)."""
    nc = _get_nc()
    res = run_bass_kernel_spmd(nc, _in_maps(x, centers, y), list(range(M)), **spmd_kwargs)
    return _finalize(res.results), res


def kernel(x: np.ndarray, centers: np.ndarray, y: np.ndarray) -> np.ndarray:
    loss, _ = run(x, centers, y)
    return loss


# revision 5
# speedup vs baseline: 1.1794x; 1.0049x over previous
"""CenterLoss kernel for 8 Trainium2 NeuronCores.

The reference discards the addmm cross term, so

    loss = (1/B) * sum_i (||x_i||^2 + ||centers[y_i]||^2) + (C-1) * 1e-12

(the constant comes from clip(0, 1e-12, 1e12) applied to the B*(C-1) zero
entries of dist; the nonzero entries are ~1e2, far inside the clamp range).
The [B, C] matrix never needs to be materialized.

Data-parallel over batch: each core squares+row-sums its x slab (viewed as
[128, 2048]) and gathers its 2048 centers[y_i] rows with 4 dma_gather
instructions (512 rows each; 2048 in one overflows the SWDGE ring and
kills the NEFF), squares+row-sums those, collapses the [128,1] partial to
one element with a PE matmul against ones (a [128,1]-strided output DMA
costs ~8us in completion latency; a 1-element DMA doesn't), and writes a
scalar. The host sums the 8 partials in float64.

Raw Bass (no Tile): this toolchain's walrus rejects Tile's tail drain and
runtime-faults on TENSOR_TENSOR_REDUCE, so sync is explicit. Extended
bass_isa instructions (dma_gather, load_library) need
library_overlay.lower_extended_insts(nc) or walrus sees empty .instr bytes
("ISA wrong length"). dma_gather needs gpsimd.load_library(mlp). DVE
same-engine RAW needs drain(). A semaphore may only be waited at its full
count (partial counts race: the 16 per-SDMA-engine increments of
concurrent DMAs interleave), hence one sem per DMA / gather chunk.
"""

import numpy as np

import concourse.bass as bass
from concourse import library_config, library_overlay, mybir
from concourse.bass_utils import run_bass_kernel_spmd

B = 16384  # batch
D = 128  # feature dim
C = 10000  # num classes
M = 8  # cores
P = 128  # SBUF partitions
BPC = B // M  # 2048 batch rows per core
XFREE = BPC * D // P  # 2048 f32 per partition when x slab is viewed [P, -1]
NG = BPC // P  # 16 gathered-row groups of 128 (dest layout [P, NG, D])
NCHUNK = 4  # dma_gather instructions per core
IDXW = BPC // 16  # idx columns in the 16-partition-wrapped int16 layout

_F32 = mybir.dt.float32
_I16 = mybir.dt.int16


def _build_nc() -> bass.Bass:
    nc = bass.Bass()
    xs = nc.dram_tensor("xs", [P, XFREE], _F32, kind="ExternalInput")
    ys = nc.dram_tensor("ys", [P, IDXW], _I16, kind="ExternalInput")
    cs = nc.dram_tensor("cs", [C, D], _F32, kind="ExternalInput")
    out = nc.dram_tensor("out", [1, 1], _F32, kind="ExternalOutput")

    with (
        nc.sbuf_tensor([P, IDXW], _I16) as y_sb,
        nc.sbuf_tensor([P, NG, D], _F32) as g_sb,
        nc.sbuf_tensor([P, XFREE], _F32) as x_sb,
        nc.sbuf_tensor([P, XFREE], _F32) as tr_sb,
        nc.sbuf_tensor([P, NG, D], _F32) as trg_sb,
        nc.sbuf_tensor([P, 6], _F32) as acc_sb,
        nc.sbuf_tensor([P, 1], _F32) as ones_sb,
        nc.sbuf_tensor([1, 1], _F32) as fin_sb,
        nc.psum_tensor([1, 1], _F32) as ps,
        nc.semaphore() as ysem,
        nc.semaphore() as xsem,
        nc.semaphore() as gsem0,
        nc.semaphore() as gsem1,
        nc.semaphore() as gsem2,
        nc.semaphore() as gsem3,
        nc.semaphore() as vsem,
        nc.semaphore() as tsem,
        nc.semaphore() as csem,
        nc.semaphore() as osem,
        nc.Block() as block,
    ):
        gsems = [gsem0, gsem1, gsem2, gsem3]

        @block.sync
        def _(sync):
            sync.dma_start(out=y_sb[:], in_=ys[:]).then_inc(ysem, 16)
            sync.dma_start(out=x_sb[:], in_=xs[:]).then_inc(xsem, 16)
            sync.wait_ge(csem, 1)
            sync.dma_start(out=out[:], in_=fin_sb[:]).then_inc(osem, 16)
            sync.wait_ge(osem, 16)

        @block.gpsimd
        def _(gpsimd):
            gpsimd.load_library(library_config.mlp)
            gpsimd.wait_ge(ysem, 16)
            for c in range(NCHUNK):
                nidx = BPC // NCHUNK  # 512 rows per gather
                gpsimd.dma_gather(
                    out_ap=g_sb[:, c * (NG // NCHUNK) : (c + 1) * (NG // NCHUNK), :],
                    in_ap=cs[:],
                    idxs_ap=y_sb[:, c * (IDXW // NCHUNK) : (c + 1) * (IDXW // NCHUNK)],
                    num_idxs=nidx,
                    num_idxs_reg=nidx,
                    elem_size=D,
                ).then_inc(gsems[c], 16)

        @block.vector
        def _(vector):
            vector.memset(ones_sb[:], 1.0)
            vector.wait_ge(xsem, 16)
            vector.tensor_mul(tr_sb[:], x_sb[:], x_sb[:])
            for k in range(NCHUNK):
                vector.wait_ge(gsems[k], 16)
                sl = slice(k * (NG // NCHUNK), (k + 1) * (NG // NCHUNK))
                gf = g_sb[:, sl, :].rearrange("p a b -> p (a b)")
                tf = trg_sb[:, sl, :].rearrange("p a b -> p (a b)")
                vector.tensor_mul(tf, gf, gf)
                vector.drain()
                if k == 0:
                    vector.reduce_sum(
                        acc_sb[:, 0:1], tr_sb[:], axis=mybir.AxisListType.X
                    )
                vector.reduce_sum(
                    acc_sb[:, 1 + k : 2 + k], tf, axis=mybir.AxisListType.X
                )
            vector.drain()
            vector.reduce_sum(acc_sb[:, 5:6], acc_sb[:, 0:5], axis=mybir.AxisListType.X)
            vector.drain()
            vector.nop().then_inc(vsem, 1)
            vector.wait_ge(tsem, 1)
            vector.tensor_copy(fin_sb[:], ps[:])
            vector.drain()
            vector.nop().then_inc(csem, 1)

        @block.tensor
        def _(tensor):
            tensor.wait_ge(vsem, 1)
            nc.tensor.matmul(
                ps[:], lhsT=acc_sb[:, 5:6], rhs=ones_sb[:], start=True, stop=True
            ).then_inc(tsem, 1)

    library_overlay.lower_extended_insts(nc)
    return nc


_NC_CACHE: list = []


def _get_nc() -> bass.Bass:
    if not _NC_CACHE:
        _NC_CACHE.append(_build_nc())
    return _NC_CACHE[0]


def _in_maps(x: np.ndarray, centers: np.ndarray, y: np.ndarray) -> list[dict]:
    x = np.ascontiguousarray(np.asarray(x, dtype=np.float32))
    centers = np.ascontiguousarray(np.asarray(centers, dtype=np.float32))
    y64 = np.asarray(y).reshape(B)
    maps = []
    for k in range(M):
        y_slab = y64[k * BPC : (k + 1) * BPC].astype(np.int16)
        # 16-partition-wrapped dma_gather layout, replicated to all 8 Q7 cores
        blk = y_slab.reshape(IDXW, 16).T  # [16, IDXW]
        maps.append(
            {
                "xs": x[k * BPC : (k + 1) * BPC].reshape(P, XFREE),
                "ys": np.ascontiguousarray(np.tile(blk, (8, 1))),  # [128, IDXW]
                "cs": centers,
            }
        )
    return maps


def _finalize(results: list[dict]) -> np.ndarray:
    total = 0.0
    for r in results:
        total += float(np.sum(r["out"].astype(np.float64)))
    loss = total / B + (C - 1) * 1e-12
    return np.float32(loss)


def run(x, centers, y, **spmd_kwargs):
    """Run on 8 cores; returns (loss, BassKernelResults)."""
    nc = _get_nc()
    res = run_bass_kernel_spmd(nc, _in_maps(x, centers, y), list(range(M)), **spmd_kwargs)
    return _finalize(res.results), res


def kernel(x: np.ndarray, centers: np.ndarray, y: np.ndarray) -> np.ndarray:
    loss, _ = run(x, centers, y)
    return loss


# revision 6
# speedup vs baseline: 1.2925x; 1.0959x over previous
"""CenterLoss kernel for 8 Trainium2 NeuronCores.

The reference discards the addmm cross term, so

    loss = (1/B) * sum_i (||x_i||^2 + ||centers[y_i]||^2) + (C-1) * 1e-12

(the constant comes from clip(0, 1e-12, 1e12) applied to the B*(C-1) zero
entries of dist; the nonzero entries are ~1e2, far inside the clamp range).
The [B, C] matrix never needs to be materialized.

Data-parallel over batch: each core squares+row-sums its x slab (viewed as
[128, 2048]) and gathers its 2048 centers[y_i] rows with 16 indirect DMAs
(128 rows each, one row per partition; multi-offset-per-partition forms
mis-gather ~0.5% of rows on HW), squares+row-sums those, collapses the [128,1] partial to
one element with a PE matmul against ones (a [128,1]-strided output DMA
costs ~8us in completion latency; a 1-element DMA doesn't), and writes a
scalar. The host sums the 8 partials in float64. The gather wall is SWDGE
descriptor generation on the Q7 (~9ns/descriptor, ~19us for 2048 rows);
dma_gather batches descriptors but its mlp-library load gates the stream
~8us later, a wash -- 16 plain indirect DMAs start earlier and win.

Raw Bass (no Tile): this toolchain's walrus rejects Tile's tail drain and
runtime-faults on TENSOR_TENSOR_REDUCE, so sync is explicit. Extended
bass_isa instructions (dma_gather, load_library) need
library_overlay.lower_extended_insts(nc) or walrus sees empty .instr bytes
("ISA wrong length"). dma_gather needs gpsimd.load_library(mlp). DVE
same-engine RAW needs drain(). A semaphore may only be waited at its full
count (partial counts race: the 16 per-SDMA-engine increments of
concurrent DMAs interleave), hence one sem per DMA / gather chunk.
"""

import numpy as np

import concourse.bass as bass
from concourse import library_overlay, mybir
from concourse.bass_utils import run_bass_kernel_spmd

B = 16384  # batch
D = 128  # feature dim
C = 10000  # num classes
M = 8  # cores
P = 128  # SBUF partitions
BPC = B // M  # 2048 batch rows per core
XFREE = BPC * D // P  # 2048 f32 per partition when x slab is viewed [P, -1]
NG = BPC // P  # 16 gathered-row groups of 128 (dest layout [P, NG, D])
NCHUNK = 4  # gather semaphore groups (full-count waits only)
GW = NG // NCHUNK

_F32 = mybir.dt.float32
_I32 = mybir.dt.int32


def _build_nc() -> bass.Bass:
    nc = bass.Bass()
    xs = nc.dram_tensor("xs", [P, XFREE], _F32, kind="ExternalInput")
    ys = nc.dram_tensor("ys", [P, NG], _I32, kind="ExternalInput")
    cs = nc.dram_tensor("cs", [C, D], _F32, kind="ExternalInput")
    out = nc.dram_tensor("out", [1, 1], _F32, kind="ExternalOutput")

    with (
        nc.sbuf_tensor([P, NG], _I32) as y_sb,
        nc.sbuf_tensor([P, NG, D], _F32) as g_sb,
        nc.sbuf_tensor([P, XFREE], _F32) as x_sb,
        nc.sbuf_tensor([P, XFREE], _F32) as tr_sb,
        nc.sbuf_tensor([P, NG, D], _F32) as trg_sb,
        nc.sbuf_tensor([P, 6], _F32) as acc_sb,
        nc.sbuf_tensor([P, 1], _F32) as ones_sb,
        nc.sbuf_tensor([1, 1], _F32) as fin_sb,
        nc.psum_tensor([1, 1], _F32) as ps,
        nc.semaphore() as ysem,
        nc.semaphore() as xsem,
        nc.semaphore() as gsem0,
        nc.semaphore() as gsem1,
        nc.semaphore() as gsem2,
        nc.semaphore() as gsem3,
        nc.semaphore() as vsem,
        nc.semaphore() as tsem,
        nc.semaphore() as csem,
        nc.semaphore() as osem,
        nc.Block() as block,
    ):
        gsems = [gsem0, gsem1, gsem2, gsem3]

        @block.sync
        def _(sync):
            sync.dma_start(out=y_sb[:], in_=ys[:]).then_inc(ysem, 16)
            sync.dma_start(out=x_sb[:], in_=xs[:]).then_inc(xsem, 16)
            sync.wait_ge(csem, 1)
            sync.dma_start(out=out[:], in_=fin_sb[:]).then_inc(osem, 16)
            sync.wait_ge(osem, 16)

        @block.gpsimd
        def _(gpsimd):
            gpsimd.wait_ge(ysem, 16)
            for g in range(NG):
                gpsimd.indirect_dma_start(
                    out=g_sb[:, g, :],
                    out_offset=None,
                    in_=cs[:],
                    in_offset=bass.IndirectOffsetOnAxis(ap=y_sb[:, g : g + 1], axis=0),
                ).then_inc(gsems[g // GW], 16)

        @block.vector
        def _(vector):
            vector.memset(ones_sb[:], 1.0)
            vector.wait_ge(xsem, 16)
            vector.tensor_mul(tr_sb[:], x_sb[:], x_sb[:])
            for k in range(NCHUNK):
                vector.wait_ge(gsems[k], 16 * GW)
                sl = slice(k * GW, (k + 1) * GW)
                gf = g_sb[:, sl, :].rearrange("p a b -> p (a b)")
                tf = trg_sb[:, sl, :].rearrange("p a b -> p (a b)")
                vector.tensor_mul(tf, gf, gf)
                vector.drain()
                if k == 0:
                    vector.reduce_sum(
                        acc_sb[:, 0:1], tr_sb[:], axis=mybir.AxisListType.X
                    )
                vector.reduce_sum(
                    acc_sb[:, 1 + k : 2 + k], tf, axis=mybir.AxisListType.X
                )
            vector.drain()
            vector.reduce_sum(acc_sb[:, 5:6], acc_sb[:, 0:5], axis=mybir.AxisListType.X)
            vector.drain()
            vector.nop().then_inc(vsem, 1)
            vector.wait_ge(tsem, 1)
            vector.tensor_copy(fin_sb[:], ps[:])
            vector.drain()
            vector.nop().then_inc(csem, 1)

        @block.tensor
        def _(tensor):
            tensor.wait_ge(vsem, 1)
            nc.tensor.matmul(
                ps[:], lhsT=acc_sb[:, 5:6], rhs=ones_sb[:], start=True, stop=True
            ).then_inc(tsem, 1)

    library_overlay.lower_extended_insts(nc)
    return nc


_NC_CACHE: list = []


def _get_nc() -> bass.Bass:
    if not _NC_CACHE:
        _NC_CACHE.append(_build_nc())
    return _NC_CACHE[0]


def _in_maps(x: np.ndarray, centers: np.ndarray, y: np.ndarray) -> list[dict]:
    x = np.ascontiguousarray(np.asarray(x, dtype=np.float32))
    centers = np.ascontiguousarray(np.asarray(centers, dtype=np.float32))
    y64 = np.asarray(y).reshape(B)
    maps = []
    for k in range(M):
        maps.append(
            {
                "xs": x[k * BPC : (k + 1) * BPC].reshape(P, XFREE),
                "ys": np.ascontiguousarray(
                    y64[k * BPC : (k + 1) * BPC].astype(np.int32).reshape(P, NG)
                ),
                "cs": centers,
            }
        )
    return maps


def _finalize(results: list[dict]) -> np.ndarray:
    total = 0.0
    for r in results:
        total += float(np.sum(r["out"].astype(np.float64)))
    loss = total / B + (C - 1) * 1e-12
    return np.float32(loss)


def run(x, centers, y, **spmd_kwargs):
    """Run on 8 cores; returns (loss, BassKernelResults)."""
    nc = _get_nc()
    res = run_bass_kernel_spmd(nc, _in_maps(x, centers, y), list(range(M)), **spmd_kwargs)
    return _finalize(res.results), res


def kernel(x: np.ndarray, centers: np.ndarray, y: np.ndarray) -> np.ndarray:
    loss, _ = run(x, centers, y)
    return loss
